# revision 1
# baseline (speedup 1.0000x reference)
"""Trainium2 Bass kernel for the LSTM encoder (B=64, T=512, D=128, H=512, DS=32).

Strategy: data-parallel over batch across 8 NeuronCores (8 rows each);
LSTM + dense computed fully on-chip per core.

Layout: the recurrence runs "transposed" — gate pre-activations z^T live as
[128 partitions = gate-dim chunk, chunk x batch] so the elementwise chain uses
all 128 partitions. Gate columns are permuted to chunk order [i, o, f, g] so
sigmoid/tanh apply to contiguous column ranges. Matmul inputs are fp16
(accumulation fp32 in PSUM); state/c/h chain is fp32, h stored fp16 for the
next-step matmul and the dense layer.
"""

import os
import numpy as np

import concourse.bass as bass
import concourse.tile as tile
from concourse import bacc, mybir
from concourse.bass_utils import run_bass_kernel_spmd

B, T, D, H, DS = 64, 512, 128, 512, 32
NCORES = 8
BL = B // NCORES          # local batch rows per core
KC = H // 128             # k-chunks of the hidden dim
GC = 4 * H // 128         # gate-dim chunks (16)
HGC = GC // 2             # chunks per psum half (8)
F16 = mybir.dt.float16
F32 = mybir.dt.float32

# permuted gate-column order: i, o, f, g (so each ACT op covers contiguous chunks)
_ORDER = np.concatenate([
    np.arange(0, H),           # i
    np.arange(3 * H, 4 * H),   # o
    np.arange(H, 2 * H),       # f
    np.arange(2 * H, 3 * H),   # g
])

LAST_RUN = {}


def _const_or_none(vals):
    v = float(vals.flat[0])
    return v if np.all(vals == v) else None


def _build(bias_perm, n_steps=T):
    """Build the per-core SPMD program. bias_perm: permuted bias [4H] fp32."""
    nc = bacc.Bacc("TRN2", target_bir_lowering=False, debug=False,
                   num_devices=NCORES)

    xt_d = nc.dram_tensor("xt", [128, n_steps, BL], F16, kind="ExternalInput")
    w_d = nc.dram_tensor("w", [128, GC, 128], F16, kind="ExternalInput")
    r_d = nc.dram_tensor("r", [128, KC, GC, 128], F16, kind="ExternalInput")
    wd_d = nc.dram_tensor("wd", [128, KC, DS], F16, kind="ExternalInput")
    biast_d = nc.dram_tensor("biasT", [128, GC], F32, kind="ExternalInput")
    db_d = nc.dram_tensor("db", [DS, 1], F32, kind="ExternalInput")
    outt_d = nc.dram_tensor("outT", [DS, n_steps, BL], F32, kind="ExternalOutput")
    ht_d = nc.dram_tensor("hT", [128, KC * BL], F32, kind="ExternalOutput")
    ct_d = nc.dram_tensor("cT", [128, KC * BL], F32, kind="ExternalOutput")

    CB = KC * BL  # columns of one gate block = 32

    # bias constants per ACT group (None -> per-chunk AP bias fallback)
    bias_chunks = bias_perm.reshape(GC, 128)
    c_io = _const_or_none(bias_chunks[0:2 * KC])
    c_f = _const_or_none(bias_chunks[2 * KC:3 * KC])
    c_g = _const_or_none(bias_chunks[3 * KC:4 * KC])

    SIG = mybir.ActivationFunctionType.Sigmoid
    TANH = mybir.ActivationFunctionType.Tanh

    with tile.TileContext(nc) as tc:
        with tc.tile_pool(name="persist", bufs=1) as pp, \
             tc.tile_pool(name="work", bufs=3) as wp, \
             tc.tile_pool(name="psA", bufs=2, space="PSUM") as psA, \
             tc.tile_pool(name="psB", bufs=2, space="PSUM") as psB, \
             tc.tile_pool(name="psD", bufs=2, space="PSUM") as psD:

            xt = pp.tile([128, n_steps, BL], F16)
            wsb = pp.tile([128, GC, 128], F16)
            rsb = pp.tile([128, KC, GC, 128], F16)
            wdsb = pp.tile([128, KC, DS], F16)
            biast = pp.tile([128, GC], F32)
            dbsb = pp.tile([DS, 1], F32)
            hh = pp.tile([128, n_steps + 1, CB], F16)   # h^T history, slot 0 = h_{-1}=0
            ct = pp.tile([128, CB], F32)                # running c (transposed layout)

            nc.sync.dma_start(xt[:], xt_d[:])
            nc.sync.dma_start(wsb[:], w_d[:])
            nc.sync.dma_start(rsb[:], r_d[:])
            nc.sync.dma_start(wdsb[:], wd_d[:])
            nc.sync.dma_start(biast[:], biast_d[:])
            nc.sync.dma_start(dbsb[:], db_d[:])

            nc.vector.memset(hh[:, 0, :], 0.0)
            nc.vector.memset(ct[:], 0.0)

            def act(out, in_, func, const, chunk0, nchunks):
                """Activation on `nchunks` contiguous chunks with bias handling."""
                if const is not None:
                    if const == 0.0:
                        nc.scalar.activation(out, in_, func)
                    else:
                        nc.scalar.activation(out, in_, func, bias=const)
                else:
                    for cc in range(nchunks):
                        s = slice(cc * BL, (cc + 1) * BL)
                        nc.scalar.activation(out[:, s], in_[:, s], func,
                                             bias=biast[:, chunk0 + cc:chunk0 + cc + 1])

            for t in range(n_steps):
                zA = psA.tile([128, HGC * BL], F32)   # chunks 0..7  = [i | o]
                zB = psB.tile([128, HGC * BL], F32)   # chunks 8..15 = [f | g]
                for c in range(GC):
                    z = zA if c < HGC else zB
                    col = (c % HGC) * BL
                    o_ap = z[:, col:col + BL]
                    nc.tensor.matmul(o_ap, wsb[:, c, :], xt[:, t, :],
                                     start=True, stop=False)
                    for k in range(KC):
                        nc.tensor.matmul(o_ap, rsb[:, k, c, :],
                                         hh[:, t, k * BL:(k + 1) * BL],
                                         start=False, stop=(k == KC - 1))

                gA = wp.tile([128, HGC * BL], F32)    # [i | o] after sigmoid
                gB = wp.tile([128, HGC * BL], F32)    # [f | g] after sigmoid/tanh
                act(gA[:], zA[:], SIG, c_io, 0, 2 * KC)
                act(gB[:, 0:CB], zB[:, 0:CB], SIG, c_f, 2 * KC, KC)
                act(gB[:, CB:2 * CB], zB[:, CB:2 * CB], TANH, c_g, 3 * KC, KC)

                t1 = wp.tile([128, CB], F32)
                t2 = wp.tile([128, CB], F32)
                nc.vector.tensor_mul(t1[:], gA[:, 0:CB], gB[:, CB:2 * CB])  # i*g
                nc.vector.tensor_mul(t2[:], ct[:], gB[:, 0:CB])             # c*f
                nc.vector.tensor_add(ct[:], t1[:], t2[:])                   # c
                th = wp.tile([128, CB], F32)
                nc.scalar.activation(th[:], ct[:], TANH)
                nc.vector.tensor_mul(hh[:, t + 1, :], gA[:, CB:2 * CB], th[:])  # h fp16

                if t == n_steps - 1:
                    h32 = pp.tile([128, CB], F32)
                    nc.vector.tensor_mul(h32[:], gA[:, CB:2 * CB], th[:])
                    nc.sync.dma_start(ht_d[:], h32[:])
                    nc.sync.dma_start(ct_d[:], ct[:])

            # dense: out^T[ds, (t, b)] = tanh(Wd^T h + db), blocks of 64 steps
            TB = 512 // BL                            # timesteps per psum bank
            for blk in range((n_steps + TB - 1) // TB):
                t0 = blk * TB
                tn = min(TB, n_steps - t0)
                pd = psD.tile([DS, TB * BL], F32)
                for k in range(KC):
                    nc.tensor.matmul(pd[:, 0:tn * BL], wdsb[:, k, :],
                                     hh[:, 1 + t0:1 + t0 + tn, k * BL:(k + 1) * BL],
                                     start=(k == 0), stop=(k == KC - 1))
                od = wp.tile([DS, TB * BL], F32)
                nc.scalar.activation(od[:, 0:tn * BL], pd[:, 0:tn * BL], TANH,
                                     bias=dbsb[:, 0:1])
                nc.sync.dma_start(outt_d[:, t0:t0 + tn, :], od[:, 0:tn * BL])

    nc.compile()
    return nc


def _prep_shared(kernel, rec_kernel, bias, dense_w, dense_b):
    wp = np.ascontiguousarray(kernel[:, _ORDER].astype(np.float16)
                              .reshape(128, GC, 128))
    rp = np.ascontiguousarray(rec_kernel[:, _ORDER].astype(np.float16)
                              .reshape(KC, 128, GC, 128).transpose(1, 0, 2, 3))
    wdp = np.ascontiguousarray(dense_w.astype(np.float16)
                               .reshape(KC, 128, DS).transpose(1, 0, 2))
    biast = np.ascontiguousarray(bias[_ORDER].astype(np.float32)
                                 .reshape(GC, 128).T)
    db = np.ascontiguousarray(dense_b.astype(np.float32).reshape(DS, 1))
    return wp, rp, wdp, biast, db


def kernel(x, kernel, rec_kernel, bias, dense_w, dense_b):
    x = np.asarray(x, dtype=np.float32)
    bias_perm = np.asarray(bias, dtype=np.float32)[_ORDER]
    nc = _build(bias_perm)

    wp, rp, wdp, biast, db = _prep_shared(
        np.asarray(kernel, np.float32), np.asarray(rec_kernel, np.float32),
        np.asarray(bias, np.float32), np.asarray(dense_w, np.float32),
        np.asarray(dense_b, np.float32))

    x16 = x.astype(np.float16)
    in_maps = []
    for j in range(NCORES):
        xs = np.ascontiguousarray(x16[j * BL:(j + 1) * BL].transpose(2, 1, 0))
        in_maps.append({"xt": xs, "w": wp, "r": rp, "wd": wdp,
                        "biasT": biast, "db": db})

    trace = bool(int(os.environ.get("LSTM_KERNEL_TRACE", "0")))
    res = run_bass_kernel_spmd(nc, in_maps, core_ids=list(range(NCORES)),
                               trace=trace)
    LAST_RUN.clear()
    LAST_RUN["exec_time_ns"] = res.exec_time_ns
    LAST_RUN["profile_json"] = res.profile_json

    outs, hs, cs = [], [], []
    for j in range(NCORES):
        r = res.results[j]
        outs.append(np.ascontiguousarray(r["outT"].transpose(2, 1, 0)))
        hs.append(r["hT"].reshape(128, KC, BL).transpose(2, 1, 0).reshape(BL, H))
        cs.append(r["cT"].reshape(128, KC, BL).transpose(2, 1, 0).reshape(BL, H))
    out = np.concatenate(outs, axis=0).astype(np.float32)
    h = np.concatenate(hs, axis=0).astype(np.float32)
    c = np.concatenate(cs, axis=0).astype(np.float32)
    return out, h, c


# revision 4
# speedup vs baseline: 1.2542x; 1.2542x over previous
"""Trainium2 Bass kernel for the LSTM encoder (B=64, T=512, D=128, H=512, DS=32).

Strategy: data-parallel over batch across 8 NeuronCores (8 rows each);
LSTM + dense computed fully on-chip per core.

Layout: the recurrence runs "transposed" — gate pre-activations z^T live as
[128 partitions = gate-dim chunk, chunk x batch] so the elementwise chain uses
all 128 partitions. Gate columns are permuted to chunk order [i, o, f, g] so
sigmoid/tanh apply to contiguous column ranges. Matmul inputs are fp16
(accumulation fp32 in PSUM); state/c/h chain is fp32, h stored fp16 for the
next-step matmul and the dense layer.
"""

import os
import numpy as np

import concourse.bass as bass
import concourse.tile as tile
from concourse import bacc, mybir
from concourse.bass_utils import run_bass_kernel_spmd

B, T, D, H, DS = 64, 512, 128, 512, 32
NCORES = 8
BL = B // NCORES          # local batch rows per core
KC = H // 128             # k-chunks of the hidden dim
GC = 4 * H // 128         # gate-dim chunks (16)
HGC = GC // 2             # chunks per psum half (8)
F16 = mybir.dt.float16
F32 = mybir.dt.float32

# permuted gate-column order: i, o, f, g (so each ACT op covers contiguous chunks)
_ORDER = np.concatenate([
    np.arange(0, H),           # i
    np.arange(3 * H, 4 * H),   # o
    np.arange(H, 2 * H),       # f
    np.arange(2 * H, 3 * H),   # g
])

LAST_RUN = {}


def _const_or_none(vals):
    v = float(vals.flat[0])
    return v if np.all(vals == v) else None


def _build(bias_perm, n_steps=T):
    """Build the per-core SPMD program. bias_perm: permuted bias [4H] fp32."""
    nc = bacc.Bacc("TRN2", target_bir_lowering=False, debug=False,
                   num_devices=NCORES)

    xt_d = nc.dram_tensor("xt", [128, n_steps, BL], F16, kind="ExternalInput")
    w_d = nc.dram_tensor("w", [128, GC, 128], F16, kind="ExternalInput")
    r_d = nc.dram_tensor("r", [128, KC, GC, 128], F16, kind="ExternalInput")
    wd_d = nc.dram_tensor("wd", [128, KC, DS], F16, kind="ExternalInput")
    biast_d = nc.dram_tensor("biasT", [128, GC], F32, kind="ExternalInput")
    db_d = nc.dram_tensor("db", [DS, 1], F32, kind="ExternalInput")
    outt_d = nc.dram_tensor("outT", [DS, n_steps, BL], F32, kind="ExternalOutput")
    ht_d = nc.dram_tensor("hT", [128, KC * BL], F32, kind="ExternalOutput")
    ct_d = nc.dram_tensor("cT", [128, KC * BL], F32, kind="ExternalOutput")

    CB = KC * BL  # columns of one gate block = 32

    # bias constants per ACT group (None -> per-chunk AP bias fallback)
    bias_chunks = bias_perm.reshape(GC, 128)
    c_io = _const_or_none(bias_chunks[0:2 * KC])
    c_f = _const_or_none(bias_chunks[2 * KC:3 * KC])
    c_g = _const_or_none(bias_chunks[3 * KC:4 * KC])

    SIG = mybir.ActivationFunctionType.Sigmoid
    TANH = mybir.ActivationFunctionType.Tanh

    # gate -> chunk range in permuted order [i(0:4), o(4:8), f(8:12), g(12:16)]
    GATES = {"i": 0, "o": KC, "f": 2 * KC, "g": 3 * KC}
    consts = {"i": _const_or_none(bias_chunks[0:KC]),
              "o": _const_or_none(bias_chunks[KC:2 * KC]),
              "f": c_f, "g": c_g}

    with tile.TileContext(nc) as tc:
        with tc.tile_pool(name="persist", bufs=1) as pp, \
             tc.tile_pool(name="work", bufs=3) as wp:

            xt = pp.tile([128, n_steps, BL], F16)
            wsb = pp.tile([128, GC, 128], F16)
            rsb = pp.tile([128, KC, GC, 128], F16)
            wdsb = pp.tile([128, KC, DS], F16)
            biast = pp.tile([128, GC], F32)
            dbsb = pp.tile([DS, 1], F32)
            hh = pp.tile([128, n_steps + 1, CB], F16)   # h^T history, slot 0 = h_{-1}=0
            ct = pp.tile([128, CB], F32)                # running c (transposed layout)

            nc.sync.dma_start(xt[:], xt_d[:])
            nc.sync.dma_start(wsb[:], w_d[:])
            nc.sync.dma_start(rsb[:], r_d[:])
            nc.sync.dma_start(wdsb[:], wd_d[:])
            nc.sync.dma_start(biast[:], biast_d[:])
            nc.sync.dma_start(dbsb[:], db_d[:])

            nc.vector.memset(hh[:, 0, :], 0.0)
            nc.vector.memset(ct[:], 0.0)

            def act(out, in_, func, gate):
                """Activation over one gate's [128, CB] block with bias handling."""
                const = consts[gate]
                if const is not None:
                    if const == 0.0:
                        nc.scalar.activation(out, in_, func)
                    else:
                        nc.scalar.activation(out, in_, func, bias=const)
                else:
                    for cc in range(KC):
                        s = slice(cc * BL, (cc + 1) * BL)
                        nc.scalar.activation(out[:, s], in_[:, s], func,
                                             bias=biast[:, GATES[gate] + cc:GATES[gate] + cc + 1])

            with tc.tile_pool(name="psz", bufs=2, space="PSUM") as psz:

                def w_pairs(t):
                    """Input-projection pairs for step t into fresh z tiles.
                    These only need xt, so in the PE queue they fill the window
                    where the previous step's elementwise chain runs."""
                    z = {gate: psz.tile([128, CB], F32, tag="z" + gate,
                                        name="z" + gate)
                         for gate in ("i", "o", "f", "g")}
                    for gate in ("g", "f", "i", "o"):
                        c0 = GATES[gate]
                        for cc in range(KC):
                            nc.tensor.matmul(z[gate][:, cc * BL:(cc + 1) * BL],
                                             wsb[:, c0 + cc, :], xt[:, t, :],
                                             start=True, stop=False)
                    return z

                for t in range(n_steps):
                    z = {gate: psz.tile([128, CB], F32, tag="z" + gate,
                                        name="z" + gate)
                         for gate in ("i", "o", "f", "g")}
                    for gate in ("g", "f", "i", "o"):
                        c0 = GATES[gate]
                        for cc in range(KC):
                            o_ap = z[gate][:, cc * BL:(cc + 1) * BL]
                            nc.tensor.matmul(o_ap, wsb[:, c0 + cc, :], xt[:, t, :],
                                             start=True, stop=False)
                            for k in range(KC):
                                nc.tensor.matmul(o_ap, rsb[:, k, c0 + cc, :],
                                                 hh[:, t, k * BL:(k + 1) * BL],
                                                 start=False, stop=(k == KC - 1))
                    zt = z

                    # elementwise chain; ACT order g,f,i,o then tanh(c)
                    gg = wp.tile([128, CB], F32)
                    gf = wp.tile([128, CB], F32)
                    gi = wp.tile([128, CB], F32)
                    go = wp.tile([128, CB], F32)
                    act(gg[:], zt["g"][:], TANH, "g")
                    act(gf[:], zt["f"][:], SIG, "f")
                    t2 = wp.tile([128, CB], F32)
                    nc.vector.tensor_mul(t2[:], ct[:], gf[:])       # c*f
                    act(gi[:], zt["i"][:], SIG, "i")
                    act(go[:], zt["o"][:], SIG, "o")
                    t1 = wp.tile([128, CB], F32)
                    nc.vector.tensor_mul(t1[:], gi[:], gg[:])       # i*g
                    nc.vector.tensor_add(ct[:], t1[:], t2[:])       # c
                    th = wp.tile([128, CB], F32)
                    nc.scalar.activation(th[:], ct[:], TANH)
                    nc.vector.tensor_mul(hh[:, t + 1, :], go[:], th[:])  # h fp16

                    if t == n_steps - 1:
                        h32 = pp.tile([128, CB], F32)
                        nc.vector.tensor_mul(h32[:], go[:], th[:])
                        nc.sync.dma_start(ht_d[:], h32[:])
                        nc.sync.dma_start(ct_d[:], ct[:])

            # dense: out^T[ds, (t, b)] = tanh(Wd^T h + db), blocks of 64 steps
            TB = 512 // BL                            # timesteps per psum bank
            with tc.tile_pool(name="psd", bufs=2, space="PSUM") as psD:
                for blk in range((n_steps + TB - 1) // TB):
                    t0 = blk * TB
                    tn = min(TB, n_steps - t0)
                    pd = psD.tile([DS, TB * BL], F32)
                    for k in range(KC):
                        nc.tensor.matmul(pd[:, 0:tn * BL], wdsb[:, k, :],
                                         hh[:, 1 + t0:1 + t0 + tn, k * BL:(k + 1) * BL],
                                         start=(k == 0), stop=(k == KC - 1))
                    od = wp.tile([DS, TB * BL], F32)
                    nc.scalar.activation(od[:, 0:tn * BL], pd[:, 0:tn * BL], TANH,
                                         bias=dbsb[:, 0:1])
                    nc.sync.dma_start(outt_d[:, t0:t0 + tn, :], od[:, 0:tn * BL])

    nc.compile()
    return nc


def _prep_shared(kernel, rec_kernel, bias, dense_w, dense_b):
    wp = np.ascontiguousarray(kernel[:, _ORDER].astype(np.float16)
                              .reshape(128, GC, 128))
    rp = np.ascontiguousarray(rec_kernel[:, _ORDER].astype(np.float16)
                              .reshape(KC, 128, GC, 128).transpose(1, 0, 2, 3))
    wdp = np.ascontiguousarray(dense_w.astype(np.float16)
                               .reshape(KC, 128, DS).transpose(1, 0, 2))
    biast = np.ascontiguousarray(bias[_ORDER].astype(np.float32)
                                 .reshape(GC, 128).T)
    db = np.ascontiguousarray(dense_b.astype(np.float32).reshape(DS, 1))
    return wp, rp, wdp, biast, db


def kernel(x, kernel, rec_kernel, bias, dense_w, dense_b):
    x = np.asarray(x, dtype=np.float32)
    bias_perm = np.asarray(bias, dtype=np.float32)[_ORDER]
    nc = _build(bias_perm)

    wp, rp, wdp, biast, db = _prep_shared(
        np.asarray(kernel, np.float32), np.asarray(rec_kernel, np.float32),
        np.asarray(bias, np.float32), np.asarray(dense_w, np.float32),
        np.asarray(dense_b, np.float32))

    x16 = x.astype(np.float16)
    in_maps = []
    for j in range(NCORES):
        xs = np.ascontiguousarray(x16[j * BL:(j + 1) * BL].transpose(2, 1, 0))
        in_maps.append({"xt": xs, "w": wp, "r": rp, "wd": wdp,
                        "biasT": biast, "db": db})

    trace = bool(int(os.environ.get("LSTM_KERNEL_TRACE", "0")))
    res = run_bass_kernel_spmd(nc, in_maps, core_ids=list(range(NCORES)),
                               trace=trace)
    LAST_RUN.clear()
    LAST_RUN["exec_time_ns"] = res.exec_time_ns
    LAST_RUN["profile_json"] = res.profile_json

    outs, hs, cs = [], [], []
    for j in range(NCORES):
        r = res.results[j]
        outs.append(np.ascontiguousarray(r["outT"].transpose(2, 1, 0)))
        hs.append(r["hT"].reshape(128, KC, BL).transpose(2, 1, 0).reshape(BL, H))
        cs.append(r["cT"].reshape(128, KC, BL).transpose(2, 1, 0).reshape(BL, H))
    out = np.concatenate(outs, axis=0).astype(np.float32)
    h = np.concatenate(hs, axis=0).astype(np.float32)
    c = np.concatenate(cs, axis=0).astype(np.float32)
    return out, h, c
